# revision 1
# baseline (speedup 1.0000x reference)
"""GATv2 layer on 8 Trainium2 NeuronCores (Bass/Tile).

Reference math (per batch b):
    hp = h @ lin_w.T + lin_b
    u  = hp @ W1.T ; v = hp @ W2.T          (W1, W2 = halves of W_w)
    e[i,j]   = sum_f a_f * LeakyReLU(u[i,f] + v[j,f])
    att      = softmax_j(where(adj, e, -inf))
    out      = elu(att @ hp)

Kernel decomposition:
  a_f*LReLU(s) = alpha*a_f*s + (1-alpha)*sign(a_f)*relu(|a_f|*s), so with
  u'' = |a|*u, v'' = |a|*v:
    e[i,j] = alpha*su_i + alpha*sv_j + (1-alpha) * sum_f sign(a_f)*relu(u''[i,f]+v''[j,f])
  The alpha*su_i row term cancels in the softmax ratio; exp(alpha*sv_j) is
  folded into the adjacency mask host-side (w_j).  On device, per pair of
  destination rows (i0,i1) one [128,1024] tile
      T = relu(Vstack + ubias_col)       (Vstack = v''^T stacked twice)
  is contracted by the PE with a +-1 sign matrix into two rows of e
  (accumulated into its 64-row PSUM half through one of 32 column-shifted
  sign-matrix variants, since PSUM matmul bases are restricted to {0,32,64}).
  exp(0.8*e) via ACT (scale folds (1-alpha)), masked by w_j*adj^T during the
  PSUM->SBUF copy after a PE transpose, then the PV matmul (attT @ [hp, 1])
  yields numerator and denominator in one pass; divide + ELU epilogue
  (elu(x) = relu(x) + exp(min(x, 0)) - 1).

  The PE path runs in fp16 (fp32 matmul is 1/4 rate on TRN2); e accumulates
  in fp32 PSUM.  Measured end-to-end rel err vs the fp32 reference: 2.6e-4.
  TimelineSim cost model: ~128 us/core; TensorE busy ~113 us (rhs-ingest
  bound: 256 pairs x 1024 j-columns at 128 rows/cycle @ 2.4 GHz).

Sharding: core c owns batch c//2, destination rows (c%2)*512 ... +512.
"""

import sys

import numpy as np

if "/opt/trn_rl_repo" not in sys.path:
    sys.path.insert(0, "/opt/trn_rl_repo")

ALPHA = 0.2
B, N, F = 4, 1024, 64
N_CORES = 8
ROWS_PER_CORE = B * N // N_CORES          # 512
BLK = 128
N_BLOCKS = ROWS_PER_CORE // BLK           # 4
PAIRS_PER_BLOCK = BLK // 2                # 64
N_PAIRS = ROWS_PER_CORE // 2              # 256
N_JB = N // BLK                           # 8

_COMPILED = {}


def _build_module():
    import concourse.tile as tile
    from concourse import bacc, mybir
    from contextlib import ExitStack

    f32 = mybir.dt.float32
    f16 = mybir.dt.float16
    nc = bacc.Bacc("TRN2", target_bir_lowering=False, debug=False,
                   enable_asserts=True, num_devices=N_CORES)

    vstack_ap = nc.dram_tensor("vstack", (BLK, N), f16, kind="ExternalInput").ap()
    # ubias split: block 0's 64 bias columns ship first (32 KB) so the first
    # relu pass isn't gated on the full bias transfer
    ubias0_ap = nc.dram_tensor("ubias0", (BLK, PAIRS_PER_BLOCK), f32, kind="ExternalInput").ap()
    ubias_ap = nc.dram_tensor("ubias", (BLK, N_PAIRS - PAIRS_PER_BLOCK), f32, kind="ExternalInput").ap()
    # 32 sign-matrix variants [128, 64]: variant v has the two +-sign columns
    # at 2v, 2v+1 (PE matmul PSUM output base must be in {0, 32, 64}, so a
    # pair accumulates into its 64-row half through variant v = q % 32).
    # Shipped compact ([128, 32*2]) and scattered into a zeroed tile at
    # column stride 66 (= 64 + 2) on device.
    # shipped compact; scattered on device to columns 66*v + {0,1} of a
    # zeroed [128, 2048] buffer (variant v slice starts at column 64*v, its
    # sign columns sit at within-slice offset 2*v -> absolute 66*v)
    sgn_ap = nc.dram_tensor("sgn", (BLK, 32, 2), f16, kind="ExternalInput").ap()
    ident_ap = nc.dram_tensor("ident", (BLK, BLK), f16, kind="ExternalInput").ap()
    # adjwt / hpx are host-permuted so each lands in one [128, *] SBUF tile:
    # adjwt[p, jb*512 + i] = w_j * adj[i, j],  j = jb*128 + p
    # hpx[p, jb*65 + n]    = [hp | 1][j, n],   j = jb*128 + p
    adjwt_ap = nc.dram_tensor("adjwt", (BLK, N_JB * ROWS_PER_CORE), f16, kind="ExternalInput").ap()
    hpx_ap = nc.dram_tensor("hpx", (BLK, N_JB * (F + 1)), f16, kind="ExternalInput").ap()
    out_ap = nc.dram_tensor("out", (ROWS_PER_CORE, F), f32, kind="ExternalOutput").ap()

    Relu = mybir.ActivationFunctionType.Relu
    Exp = mybir.ActivationFunctionType.Exp
    add = mybir.AluOpType.add
    amax = mybir.AluOpType.max
    amin = mybir.AluOpType.min
    mult = mybir.AluOpType.mult

    with tile.TileContext(nc) as tc, ExitStack() as ctx:
        consts = ctx.enter_context(tc.tile_pool(name="consts", bufs=1))
        tpool = ctx.enter_context(tc.tile_pool(name="tpool", bufs=11))
        epool = ctx.enter_context(tc.tile_pool(name="epool", bufs=2))
        apool = ctx.enter_context(tc.tile_pool(name="apool", bufs=3))
        spool = ctx.enter_context(tc.tile_pool(name="spool", bufs=4))
        ps_e = ctx.enter_context(tc.tile_pool(name="ps_e", bufs=2, space="PSUM"))
        ps_t = ctx.enter_context(tc.tile_pool(name="ps_t", bufs=3, space="PSUM"))
        ps_h = ctx.enter_context(tc.tile_pool(name="ps_h", bufs=1, space="PSUM"))

        ubias0 = consts.tile([BLK, PAIRS_PER_BLOCK], f32, tag="ubias0")
        nc.sync.dma_start(ubias0[:], ubias0_ap[:])
        vstack = consts.tile([BLK, N], f16, tag="vstack")
        nc.sync.dma_start(vstack[:], vstack_ap[:])
        ubias = consts.tile([BLK, N_PAIRS - PAIRS_PER_BLOCK], f32, tag="ubias")
        nc.sync.dma_start(ubias[:], ubias_ap[:])
        sgnc = consts.tile([BLK, 64], f16, tag="sgnc")
        nc.scalar.dma_start(sgnc[:], sgn_ap[:].rearrange("p v c -> p (v c)"))
        sgn = consts.tile([BLK, 64 * 32], f16, tag="sgn")
        nc.vector.memset(sgn[:], 0.0)
        sgn_pairs = sgn[:].rearrange("p (k c) -> p k c", c=2)
        nc.vector.tensor_copy(
            sgn_pairs[:, 0:1024:33, :],
            sgnc[:].rearrange("p (v c) -> p v c", c=2))
        adjwt = []
        hpx = []
        ident = []

        def load_aux():
            # issued after block 0's relu/matmul stream is underway so the
            # early compute waits don't entangle with these bulk transfers
            ident_t = consts.tile([BLK, BLK], f16, tag="ident")
            nc.gpsimd.dma_start(ident_t[:], ident_ap[:])
            ident.append(ident_t)
            adjwt_t = consts.tile([BLK, N_JB * ROWS_PER_CORE], f16, tag="adjwt")
            nc.gpsimd.dma_start(adjwt_t[:], adjwt_ap[:])
            hpx_t = consts.tile([BLK, N_JB * (F + 1)], f16, tag="hpx")
            nc.gpsimd.dma_start(hpx_t[:], hpx_ap[:])
            for jb in range(N_JB):
                adjwt.append(adjwt_t[:, jb * ROWS_PER_CORE:(jb + 1) * ROWS_PER_CORE])
                hpx.append(hpx_t[:, jb * (F + 1):(jb + 1) * (F + 1)])

        for blk in range(N_BLOCKS):
            e_ps = ps_e.tile([BLK, N], f32, tag="e")
            for q in range(PAIRS_PER_BLOCK):
                p = blk * PAIRS_PER_BLOCK + q
                T = tpool.tile([BLK, N], f16, tag="T")
                bias_col = (ubias0[:, p:p + 1] if p < PAIRS_PER_BLOCK
                            else ubias[:, p - PAIRS_PER_BLOCK:p - PAIRS_PER_BLOCK + 1])
                # Split the relu stream between DVE (tensor_scalar) and ACT.
                # ACT takes the first pairs of each block (DVE is busy with the
                # previous block's mask/epilogue there) plus a periodic share.
                act_relu = ((q < 3 and not (blk == 0 and q == 0))
                            or (q % 16 >= 14 and not (blk == N_BLOCKS - 1 and q >= 62)))
                if not act_relu:
                    nc.vector.tensor_scalar(
                        T[:], vstack[:], bias_col, 0.0, op0=add, op1=amax)
                else:
                    nc.scalar.activation(
                        T[:], vstack[:], Relu, bias=bias_col, scale=1.0)
                k = q // 32          # 64-row half within the i-block
                v = q % 32           # sign-matrix variant / position in group
                lhsT = sgn[:, 64 * v:64 * v + 64]
                nc.tensor.matmul(e_ps[64 * k:64 * k + 64, 0:512],
                                 lhsT, T[:, 0:512],
                                 start=(v == 0), stop=(v == 31))
                nc.tensor.matmul(e_ps[64 * k:64 * k + 64, 512:1024],
                                 lhsT, T[:, 512:1024],
                                 start=(v == 0), stop=(v == 31))
            if blk == 0:
                load_aux()
            # exp((1-alpha) * e), split in column halves so the first
            # transposes are not gated on the full pass
            exp_sb = epool.tile([BLK, N], f16, tag="exp")
            nc.scalar.activation(exp_sb[:, 0:512], e_ps[:, 0:512], Exp,
                                 scale=(1.0 - ALPHA))
            nc.scalar.activation(exp_sb[:, 512:1024], e_ps[:, 512:1024], Exp,
                                 scale=(1.0 - ALPHA))
            hnum = ps_h.tile([BLK, F + 1], f32, tag="hnum")
            for jb in range(N_JB):
                tp = ps_t.tile([BLK, BLK], f16, tag="tp")
                nc.tensor.transpose(tp[:], exp_sb[:, jb * BLK:(jb + 1) * BLK], ident[0][:])
                attT = apool.tile([BLK, BLK], f16, tag="attT")
                nc.vector.tensor_mul(
                    attT[:], tp[:], adjwt[jb][:, blk * BLK:(blk + 1) * BLK])
                nc.tensor.matmul(hnum[:], attT[:], hpx[jb],
                                 start=(jb == 0), stop=(jb == N_JB - 1))
            # epilogue: h = num/den, out = elu(h) = relu(h) + exp(min(h,0)) - 1
            rec = spool.tile([BLK, 1], f32, tag="rec")
            nc.vector.reciprocal(rec[:], hnum[:, F:F + 1])
            m_t = spool.tile([BLK, F], f32, tag="m_t")
            nc.vector.tensor_scalar(m_t[:], hnum[:, 0:F], rec[:, 0:1], 0.0,
                                    op0=mult, op1=amin)
            g_t = spool.tile([BLK, F], f32, tag="g_t")
            nc.scalar.activation(g_t[:], m_t[:], Exp)
            r_t = spool.tile([BLK, F], f32, tag="r_t")
            nc.vector.tensor_scalar(r_t[:], hnum[:, 0:F], rec[:, 0:1], 0.0,
                                    op0=mult, op1=amax)
            o2 = spool.tile([BLK, F], f32, tag="o2")
            nc.vector.scalar_tensor_tensor(
                o2[:], r_t[:], -1.0, g_t[:], op0=add, op1=add)
            nc.sync.dma_start(out_ap[blk * BLK:(blk + 1) * BLK, :], o2[:])

    nc.finalize()
    return nc


def _host_precompute(h, adj, lin_w, lin_b, W_w, a):
    """Build per-core device input dicts (all small math in float64)."""
    h64 = h.astype(np.float64)
    lin_w64 = lin_w.astype(np.float64)
    lin_b64 = lin_b.astype(np.float64)
    W1 = W_w[:, :F].astype(np.float64)
    W2 = W_w[:, F:].astype(np.float64)
    a64 = a[:, 0].astype(np.float64)

    M1 = W1 @ lin_w64
    c1 = W1 @ lin_b64
    M2 = W2 @ lin_w64
    c2 = W2 @ lin_b64
    aab = np.abs(a64)
    sgn_vec = np.sign(a64)
    ident = np.eye(BLK, dtype=np.float16)

    sgn_tile = np.zeros((BLK, 32, 2), dtype=np.float16)
    sgn_tile[0:F, :, 0] = sgn_vec[:, None]
    sgn_tile[F:BLK, :, 1] = sgn_vec[:, None]

    in_maps = []
    for c in range(N_CORES):
        b = c // 2
        r0 = (c % 2) * ROWS_PER_CORE
        hb = h64[b]                                        # [N, F]
        u = (hb @ M1.T + c1) * aab                         # u'' [N, F]
        v = (hb @ M2.T + c2) * aab                         # v'' [N, F]
        sv = v @ sgn_vec                                   # [N]
        w = np.exp(ALPHA * sv)                             # [N]
        hp = hb @ lin_w64.T + lin_b64                      # [N, F]

        vstack = np.concatenate([v.T, v.T], axis=0).astype(np.float16)
        us = u[r0:r0 + ROWS_PER_CORE]                      # [512, F]
        ubias = np.concatenate([us[0::2].T, us[1::2].T], axis=0).astype(np.float32)
        ubias0 = np.ascontiguousarray(ubias[:, :PAIRS_PER_BLOCK])
        ubias = ubias[:, PAIRS_PER_BLOCK:]
        adjwt = (adj[b, r0:r0 + ROWS_PER_CORE, :].T.astype(np.float64)
                 * w[:, None]).astype(np.float16)          # [N, 512]
        adjwt = adjwt.reshape(N_JB, BLK, ROWS_PER_CORE).transpose(1, 0, 2)
        adjwt = adjwt.reshape(BLK, N_JB * ROWS_PER_CORE)
        hpx = np.concatenate(
            [hp, np.ones((N, 1))], axis=1).astype(np.float16)  # [N, 65]
        hpx = hpx.reshape(N_JB, BLK, F + 1).transpose(1, 0, 2)
        hpx = hpx.reshape(BLK, N_JB * (F + 1))

        in_maps.append({
            "vstack": np.ascontiguousarray(vstack),
            "ubias0": ubias0,
            "ubias": np.ascontiguousarray(ubias),
            "sgn": sgn_tile,
            "adjwt": np.ascontiguousarray(adjwt),
            "hpx": np.ascontiguousarray(hpx),
            "ident": ident,
        })
    return in_maps


def kernel(h, adj, lin_w, lin_b, W_w, a):
    from concourse.bass_utils import run_bass_kernel_spmd

    h, adj, lin_w, lin_b, W_w, a = (
        np.asarray(x) for x in (h, adj, lin_w, lin_b, W_w, a))

    if "nc" not in _COMPILED:
        _COMPILED["nc"] = _build_module()
    nc = _COMPILED["nc"]

    in_maps = _host_precompute(h, adj, lin_w, lin_b, W_w, a)
    res = run_bass_kernel_spmd(nc, in_maps, core_ids=list(range(N_CORES)))

    out = np.empty((B, N, F), dtype=np.float32)
    for c in range(N_CORES):
        b = c // 2
        r0 = (c % 2) * ROWS_PER_CORE
        out[b, r0:r0 + ROWS_PER_CORE, :] = res.results[c]["out"]
    return out



# revision 6
# speedup vs baseline: 1.3088x; 1.3088x over previous
"""GATv2 layer on 8 Trainium2 NeuronCores (Bass/Tile).

Reference math (per batch b):
    hp = h @ lin_w.T + lin_b
    u  = hp @ W1.T ; v = hp @ W2.T          (W1, W2 = halves of W_w)
    e[i,j]   = sum_f a_f * LeakyReLU(u[i,f] + v[j,f])
    att      = softmax_j(where(adj, e, -inf))
    out      = elu(att @ hp)

Kernel decomposition (same algebra as the fp16 predecessor):
  a_f*LReLU(s) = alpha*a_f*s + (1-alpha)*sign(a_f)*relu(|a_f|*s); with
  u'' = |a|*u, v'' = |a|*v the alpha*su_i row term cancels in softmax and
  exp(alpha*sv_j) folds into the adjacency mask host-side.  The remaining
  work per (i, j) is the 64-term signed-relu contraction
      c[i,j] = sum_f sign(a_f) * relu(u''[i,f] + v''[j,f]).

  Mixed-precision f-split: features are ranked host-side by the second
  moment of u''+v''; the top 32 ("hot") contract in fp16, the bottom 32
  ("cold") in fp8e4m3 via DoubleRow matmuls.  Per quad of destinations a
  [128, 1024] hot tile (4 dests x 32 f stacked on partitions) feeds one
  fp16 matmul per 512-wide j-half; per octet a [128, 2048] cold tile (two
  quad k-tiles) feeds one DoubleRow matmul ([128, 2, 512] moving operand,
  256-deep contraction).  Row-shifted +-sign weight variants place each
  group's rows inside the [64, 512] PSUM half (matmul output base
  partitions are restricted to {0, 64}), so 16 hot + 8 DR matmuls
  accumulate one e half.  Relative error vs the fp32 reference: ~4e-3
  (fp8 tail features), inside the 2e-2 gate.

  Tile production: relu(vstack + ubias_col) via tensor_scalar /
  activation, split across DVE (hot fp16, 4x mode), ACT and GPSIMD (cold
  fp8) so production overlaps the PE stream.  exp(0.8*e) via ACT, masked
  by w_j*adj^T during the PSUM->SBUF copy after a PE transpose, then the
  PV matmul (attT @ [hp, 1]) yields numerator and denominator in one
  pass; divide + ELU epilogue.

Sharding: core c owns batch c//2, destination rows (c%2)*512 ... +512.
"""

import sys

import numpy as np

if "/opt/trn_rl_repo" not in sys.path:
    sys.path.insert(0, "/opt/trn_rl_repo")

ALPHA = 0.2
B, N, F = 4, 1024, 64
N_CORES = 8
ROWS_PER_CORE = B * N // N_CORES          # 512
BLK = 128
N_BLOCKS = ROWS_PER_CORE // BLK           # 4
N_JB = N // BLK                           # 8
HOT = 32                                  # fp16 features
COLD = F - HOT                            # fp8 features
QUADS_PER_HALF = 16                       # 64 dests / 4
OCTETS_PER_HALF = 8
N_QUADS = ROWS_PER_CORE // 4              # 128

_COMPILED = {}


def _cold_engines():
    """Engines for the 128 cold production instrs: 60 ACT, 48 GP, 20 DVE,
    interleaved by largest remainder so each engine's share arrives evenly."""
    quotas = {"scalar": 60.0, "gpsimd": 48.0, "vector": 20.0}
    acc = dict.fromkeys(quotas, 0.0)
    out = []
    for _ in range(128):
        for k in quotas:
            acc[k] += quotas[k] / 128.0
        pick = max(acc, key=lambda k: acc[k])
        acc[pick] -= 1.0
        out.append(pick)
    return out


_COLD_ENGINES = _cold_engines()


def _build_module():
    import concourse.tile as tile
    from concourse import bacc, mybir
    from contextlib import ExitStack

    f32 = mybir.dt.float32
    f16 = mybir.dt.float16
    f8 = mybir.dt.float8e4
    nc = bacc.Bacc("TRN2", target_bir_lowering=False, debug=False,
                   enable_asserts=True, num_devices=N_CORES)

    vsh_ap = nc.dram_tensor("vsh", (BLK, N), f16, kind="ExternalInput").ap()
    vsc_ap = nc.dram_tensor("vsc", (BLK, N), f16, kind="ExternalInput").ap()
    # per-quad bias columns, block 0 shipped first so early relu isn't gated
    ubh0_ap = nc.dram_tensor("ubh0", (BLK, 32), f32, kind="ExternalInput").ap()
    ubc0_ap = nc.dram_tensor("ubc0", (BLK, 32), f32, kind="ExternalInput").ap()
    ubh_ap = nc.dram_tensor("ubh", (BLK, N_QUADS - 32), f32, kind="ExternalInput").ap()
    ubc_ap = nc.dram_tensor("ubc", (BLK, N_QUADS - 32), f32, kind="ExternalInput").ap()
    # hot sign variants: 16 x [128, 64] f16; variant t has s_hot at rows
    # 32*d .. 32*d+31 of column 4t+d
    sgnh_ap = nc.dram_tensor("sgnh", (BLK, QUADS_PER_HALF * 64), f16,
                             kind="ExternalInput").ap()
    # DR sign variants: 8 x [128, 2, 64] f8
    sgnc_ap = nc.dram_tensor("sgnc", (BLK, OCTETS_PER_HALF * 128), f8,
                             kind="ExternalInput").ap()
    ident_ap = nc.dram_tensor("ident", (BLK, BLK), f16, kind="ExternalInput").ap()
    # adjwt / hpx host-permuted so each lands in one [128, *] SBUF tile:
    # adjwt[p, jb*512 + i] = w_j * adj[i, j],  j = jb*128 + p
    # hpx[p, jb*65 + n]    = [hp | 1][j, n],   j = jb*128 + p
    adjwt_ap = nc.dram_tensor("adjwt", (BLK, N_JB * ROWS_PER_CORE), f16,
                              kind="ExternalInput").ap()
    hpx_ap = nc.dram_tensor("hpx", (BLK, N_JB * (F + 1)), f16,
                            kind="ExternalInput").ap()
    out_ap = nc.dram_tensor("out", (ROWS_PER_CORE, F), f32, kind="ExternalOutput").ap()

    Relu = mybir.ActivationFunctionType.Relu
    Exp = mybir.ActivationFunctionType.Exp
    add = mybir.AluOpType.add
    amax = mybir.AluOpType.max
    amin = mybir.AluOpType.min
    mult = mybir.AluOpType.mult

    with tile.TileContext(nc) as tc, ExitStack() as ctx:
        consts = ctx.enter_context(tc.tile_pool(name="consts", bufs=1))
        hpool = ctx.enter_context(tc.tile_pool(name="hpool", bufs=6))
        cpool = ctx.enter_context(tc.tile_pool(name="cpool", bufs=4))
        epool = ctx.enter_context(tc.tile_pool(name="epool", bufs=2))
        apool = ctx.enter_context(tc.tile_pool(name="apool", bufs=3))
        spool = ctx.enter_context(tc.tile_pool(name="spool", bufs=4))
        ps_e = ctx.enter_context(tc.tile_pool(name="ps_e", bufs=2, space="PSUM"))
        ps_t = ctx.enter_context(tc.tile_pool(name="ps_t", bufs=3, space="PSUM"))
        ps_h = ctx.enter_context(tc.tile_pool(name="ps_h", bufs=1, space="PSUM"))

        ubh0 = consts.tile([BLK, 32], f32, tag="ubh0")
        nc.sync.dma_start(ubh0[:], ubh0_ap[:])
        ubc0 = consts.tile([BLK, 32], f32, tag="ubc0")
        nc.sync.dma_start(ubc0[:], ubc0_ap[:])
        vsh = consts.tile([BLK, N], f16, tag="vsh")
        nc.sync.dma_start(vsh[:], vsh_ap[:])
        vsc = consts.tile([BLK, N], f16, tag="vsc")
        nc.sync.dma_start(vsc[:], vsc_ap[:])
        sgnc = consts.tile([BLK, OCTETS_PER_HALF * 128], f8, tag="sgnc")
        nc.scalar.dma_start(sgnc[:], sgnc_ap[:])
        sgnh = consts.tile([BLK, QUADS_PER_HALF * 64], f16, tag="sgnh")
        nc.scalar.dma_start(sgnh[:], sgnh_ap[:])
        ubh = consts.tile([BLK, N_QUADS - 32], f32, tag="ubh")
        nc.sync.dma_start(ubh[:], ubh_ap[:])
        ubc = consts.tile([BLK, N_QUADS - 32], f32, tag="ubc")
        nc.sync.dma_start(ubc[:], ubc_ap[:])
        adjwt = []
        hpx = []
        ident = []

        def load_aux():
            # issued after block 0's relu/matmul stream is underway so the
            # early compute waits don't entangle with these bulk transfers
            ident_t = consts.tile([BLK, BLK], f16, tag="ident")
            nc.gpsimd.dma_start(ident_t[:], ident_ap[:])
            ident.append(ident_t)
            adjwt_t = consts.tile([BLK, N_JB * ROWS_PER_CORE], f16, tag="adjwt")
            nc.gpsimd.dma_start(adjwt_t[:], adjwt_ap[:])
            hpx_t = consts.tile([BLK, N_JB * (F + 1)], f16, tag="hpx")
            nc.gpsimd.dma_start(hpx_t[:], hpx_ap[:])
            for jb in range(N_JB):
                adjwt.append(adjwt_t[:, jb * ROWS_PER_CORE:(jb + 1) * ROWS_PER_CORE])
                hpx.append(hpx_t[:, jb * (F + 1):(jb + 1) * (F + 1)])

        def hbias(q):
            return (ubh0[:, q:q + 1] if q < 32 else ubh[:, q - 32:q - 31])

        def cbias(q):
            return (ubc0[:, q:q + 1] if q < 32 else ubc[:, q - 32:q - 31])

        # DoubleRow matmuls must write PSUM at partition base 0 (walrus emits
        # full-array col_grp for them), so the kernel processes 64-dest
        # blocks: every matmul output (DR, hot, transpose, PV) is base-0.
        cold_idx = 0
        for blk in range(2 * N_BLOCKS):              # 8 blocks of 64 dests
            e_ps = ps_e.tile([64, N], f32, tag="e")
            for o in range(OCTETS_PER_HALF):
                qa = blk * 16 + 2 * o                # global quad ids
                qb = qa + 1
                cold8 = cpool.tile([BLK, 2 * N], f8, tag="cold")
                for half, q in ((0, qa), (1, qb)):
                    eng = getattr(nc, _COLD_ENGINES[cold_idx])
                    cold_idx += 1
                    dst = cold8[:, half * N:(half + 1) * N]
                    if eng is nc.scalar:
                        nc.scalar.activation(dst, vsc[:], Relu,
                                             bias=cbias(q), scale=1.0)
                    else:
                        eng.tensor_scalar(dst, vsc[:], cbias(q), 0.0,
                                          op0=add, op1=amax)
                hotA = hpool.tile([BLK, N], f16, tag="hot")
                nc.vector.tensor_scalar(hotA[:], vsh[:], hbias(qa), 0.0,
                                        op0=add, op1=amax)
                hotB = hpool.tile([BLK, N], f16, tag="hot")
                nc.vector.tensor_scalar(hotB[:], vsh[:], hbias(qb), 0.0,
                                        op0=add, op1=amax)
                rhs3 = cold8[:].rearrange("p (t n) -> p t n", t=2)
                lw_dr = (sgnc[:, o * 128:(o + 1) * 128]
                         .rearrange("p (t m) -> p t m", t=2))
                lw_a = sgnh[:, (2 * o) * 64:(2 * o) * 64 + 64]
                lw_b = sgnh[:, (2 * o + 1) * 64:(2 * o + 1) * 64 + 64]
                for jh in range(2):
                    sl = slice(jh * 512, jh * 512 + 512)
                    out_sl = e_ps[:, sl]
                    nc.tensor.matmul(
                        out_sl, lw_dr, rhs3[:, :, sl],
                        start=(o == 0), stop=False,
                        perf_mode=mybir.MatmulPerfMode.DoubleRow)
                    nc.tensor.matmul(out_sl, lw_a, hotA[:, sl],
                                     start=False, stop=False)
                    nc.tensor.matmul(out_sl, lw_b, hotB[:, sl],
                                     start=False,
                                     stop=(o == OCTETS_PER_HALF - 1))
            if blk == 0:
                load_aux()
            exp_sb = epool.tile([64, N], f16, tag="exp")
            nc.scalar.activation(exp_sb[:], e_ps[:], Exp, scale=(1.0 - ALPHA))
            hnum = ps_h.tile([64, F + 1], f32, tag="hnum")
            for jb in range(N_JB):
                tp = ps_t.tile([BLK, 64], f16, tag="tp")
                nc.tensor.transpose(tp[:], exp_sb[:, jb * BLK:(jb + 1) * BLK],
                                    ident[0][0:64, 0:64])
                attT = apool.tile([BLK, 64], f16, tag="attT")
                nc.vector.tensor_mul(
                    attT[:], tp[:], adjwt[jb][:, blk * 64:(blk + 1) * 64])
                nc.tensor.matmul(hnum[:], attT[:], hpx[jb],
                                 start=(jb == 0), stop=(jb == N_JB - 1))
            # epilogue: h = num/den, out = elu(h) = relu(h) + exp(min(h,0)) - 1
            rec = spool.tile([64, 1], f32, tag="rec")
            nc.vector.reciprocal(rec[:], hnum[:, F:F + 1])
            m_t = spool.tile([64, F], f32, tag="m_t")
            nc.vector.tensor_scalar(m_t[:], hnum[:, 0:F], rec[:, 0:1], 0.0,
                                    op0=mult, op1=amin)
            g_t = spool.tile([64, F], f32, tag="g_t")
            nc.scalar.activation(g_t[:], m_t[:], Exp)
            r_t = spool.tile([64, F], f32, tag="r_t")
            nc.vector.tensor_scalar(r_t[:], hnum[:, 0:F], rec[:, 0:1], 0.0,
                                    op0=mult, op1=amax)
            o2 = spool.tile([64, F], f32, tag="o2")
            nc.vector.scalar_tensor_tensor(
                o2[:], r_t[:], -1.0, g_t[:], op0=add, op1=add)
            nc.sync.dma_start(out_ap[blk * 64:(blk + 1) * 64, :], o2[:])

    nc.finalize()
    return nc


def _host_precompute(h, adj, lin_w, lin_b, W_w, a):
    """Build per-core device input dicts (all small math in float64)."""
    import ml_dtypes
    f8 = ml_dtypes.float8_e4m3

    h64 = h.astype(np.float64)
    lin_w64 = lin_w.astype(np.float64)
    lin_b64 = lin_b.astype(np.float64)
    W1 = W_w[:, :F].astype(np.float64)
    W2 = W_w[:, F:].astype(np.float64)
    a64 = a[:, 0].astype(np.float64)

    M1 = W1 @ lin_w64
    c1 = W1 @ lin_b64
    M2 = W2 @ lin_w64
    c2 = W2 @ lin_b64
    aab = np.abs(a64)
    sgn_vec = np.sign(a64)
    ident = np.eye(BLK, dtype=np.float16)

    in_maps = []
    for c in range(N_CORES):
        b = c // 2
        r0 = (c % 2) * ROWS_PER_CORE
        hb = h64[b]                                        # [N, F]
        u = (hb @ M1.T + c1) * aab                         # u'' [N, F]
        v = (hb @ M2.T + c2) * aab                         # v'' [N, F]
        sv = v @ sgn_vec                                   # [N]
        w = np.exp(ALPHA * sv)                             # [N]
        hp = hb @ lin_w64.T + lin_b64                      # [N, F]

        # feature split by second moment of u + v
        mom = u.var(0) + v.var(0) + (u.mean(0) + v.mean(0)) ** 2
        order = np.argsort(-mom)
        hot_f, cold_f = order[:HOT], order[HOT:]
        s_hot, s_cold = sgn_vec[hot_f], sgn_vec[cold_f]

        v16 = v.astype(np.float16)
        vsh = np.tile(v16[:, hot_f].T, (4, 1)).astype(np.float16)   # [128, N]
        vsc = np.tile(v16[:, cold_f].T, (4, 1)).astype(np.float16)  # [128, N]

        # per-quad bias columns: quad q covers dests r0 + 4q + d, d = row//32
        uc = u[r0:r0 + ROWS_PER_CORE]                      # [512, F]
        ubh = np.empty((BLK, N_QUADS), dtype=np.float32)
        ubc = np.empty((BLK, N_QUADS), dtype=np.float32)
        for d in range(4):
            ubh[d * 32:(d + 1) * 32, :] = uc[d::4, :][:, hot_f].T
            ubc[d * 32:(d + 1) * 32, :] = uc[d::4, :][:, cold_f].T

        # hot sign variants: 16 x [128, 64]
        sgnh = np.zeros((BLK, QUADS_PER_HALF, 64), dtype=np.float16)
        for t in range(QUADS_PER_HALF):
            for d in range(4):
                sgnh[d * 32:(d + 1) * 32, t, 4 * t + d] = s_hot
        sgnh = sgnh.reshape(BLK, QUADS_PER_HALF * 64)

        # DR sign variants: 8 x [128, 2, 64]
        sgnc = np.zeros((BLK, OCTETS_PER_HALF, 2, 64), dtype=f8)
        for o in range(OCTETS_PER_HALF):
            for k in range(2):
                for d in range(4):
                    sgnc[d * 32:(d + 1) * 32, o, k, 8 * o + 4 * k + d] = \
                        s_cold.astype(f8)
        sgnc = sgnc.reshape(BLK, OCTETS_PER_HALF * 128)

        adjwt = (adj[b, r0:r0 + ROWS_PER_CORE, :].T.astype(np.float64)
                 * w[:, None]).astype(np.float16)          # [N, 512]
        adjwt = adjwt.reshape(N_JB, BLK, ROWS_PER_CORE).transpose(1, 0, 2)
        adjwt = adjwt.reshape(BLK, N_JB * ROWS_PER_CORE)
        hpx = np.concatenate(
            [hp, np.ones((N, 1))], axis=1).astype(np.float16)  # [N, 65]
        hpx = hpx.reshape(N_JB, BLK, F + 1).transpose(1, 0, 2)
        hpx = hpx.reshape(BLK, N_JB * (F + 1))

        in_maps.append({
            "vsh": np.ascontiguousarray(vsh),
            "vsc": np.ascontiguousarray(vsc),
            "ubh0": np.ascontiguousarray(ubh[:, :32]),
            "ubc0": np.ascontiguousarray(ubc[:, :32]),
            "ubh": np.ascontiguousarray(ubh[:, 32:]),
            "ubc": np.ascontiguousarray(ubc[:, 32:]),
            "sgnh": sgnh,
            "sgnc": sgnc,
            "adjwt": np.ascontiguousarray(adjwt),
            "hpx": np.ascontiguousarray(hpx),
            "ident": ident,
        })
    return in_maps


def kernel(h, adj, lin_w, lin_b, W_w, a):
    from concourse.bass_utils import run_bass_kernel_spmd

    h, adj, lin_w, lin_b, W_w, a = (
        np.asarray(x) for x in (h, adj, lin_w, lin_b, W_w, a))

    if "nc" not in _COMPILED:
        _COMPILED["nc"] = _build_module()
    nc = _COMPILED["nc"]

    in_maps = _host_precompute(h, adj, lin_w, lin_b, W_w, a)
    res = run_bass_kernel_spmd(nc, in_maps, core_ids=list(range(N_CORES)))

    out = np.empty((B, N, F), dtype=np.float32)
    for c in range(N_CORES):
        b = c // 2
        r0 = (c % 2) * ROWS_PER_CORE
        out[b, r0:r0 + ROWS_PER_CORE, :] = res.results[c]["out"]
    return out


# revision 10
# speedup vs baseline: 1.3774x; 1.0524x over previous
"""GATv2 layer on 8 Trainium2 NeuronCores (Bass/Tile).

Reference math (per batch b):
    hp = h @ lin_w.T + lin_b
    u  = hp @ W1.T ; v = hp @ W2.T          (W1, W2 = halves of W_w)
    e[i,j]   = sum_f a_f * LeakyReLU(u[i,f] + v[j,f])
    att      = softmax_j(where(adj, e, -inf))
    out      = elu(att @ hp)

Kernel decomposition (same algebra as the fp16 predecessor):
  a_f*LReLU(s) = alpha*a_f*s + (1-alpha)*sign(a_f)*relu(|a_f|*s); with
  u'' = |a|*u, v'' = |a|*v the alpha*su_i row term cancels in softmax and
  exp(alpha*sv_j) folds into the adjacency mask host-side.  The remaining
  work per (i, j) is the 64-term signed-relu contraction
      c[i,j] = sum_f sign(a_f) * relu(u''[i,f] + v''[j,f]).

  Mixed-precision f-split: features are ranked host-side by the second
  moment of u''+v''; the top 32 ("hot") contract in fp16, the bottom 32
  ("cold") in fp8e4m3 via DoubleRow matmuls.  Per quad of destinations a
  [128, 1024] hot tile (4 dests x 32 f stacked on partitions) feeds one
  fp16 matmul per 512-wide j-half; per octet a [128, 2048] cold tile (two
  quad k-tiles) feeds one DoubleRow matmul ([128, 2, 512] moving operand,
  256-deep contraction).  Row-shifted +-sign weight variants place each
  group's rows inside the [64, 512] PSUM half (matmul output base
  partitions are restricted to {0, 64}), so 16 hot + 8 DR matmuls
  accumulate one e half.  Relative error vs the fp32 reference: ~4e-3
  (fp8 tail features), inside the 2e-2 gate.

  Tile production: relu(vstack + ubias_col) via tensor_scalar /
  activation, split across DVE (hot fp16, 4x mode), ACT and GPSIMD (cold
  fp8) so production overlaps the PE stream.  exp(0.8*e) via ACT, masked
  by w_j*adj^T during the PSUM->SBUF copy after a PE transpose, then the
  PV matmul (attT @ [hp, 1]) yields numerator and denominator in one
  pass; divide + ELU epilogue.

Sharding: core c owns batch c//2, destination rows (c%2)*512 ... +512.
"""

import sys

import numpy as np

if "/opt/trn_rl_repo" not in sys.path:
    sys.path.insert(0, "/opt/trn_rl_repo")

ALPHA = 0.2
B, N, F = 4, 1024, 64
N_CORES = 8
ROWS_PER_CORE = B * N // N_CORES          # 512
BLK = 128
N_BLOCKS = ROWS_PER_CORE // BLK           # 4
N_JB = N // BLK                           # 8
HOT = 32                                  # fp16 features
COLD = F - HOT                            # fp8 features
QUADS_PER_HALF = 16                       # 64 dests / 4
OCTETS_PER_HALF = 8
N_QUADS = ROWS_PER_CORE // 4              # 128

_COMPILED = {}


def _cold_engines():
    """Engines for the 128 cold production instrs: 60 ACT, 48 GP, 20 DVE,
    interleaved by largest remainder so each engine's share arrives evenly."""
    quotas = {"scalar": 58.0, "gpsimd": 46.0, "vector": 24.0}
    acc = dict.fromkeys(quotas, 0.0)
    out = []
    for _ in range(128):
        for k in quotas:
            acc[k] += quotas[k] / 128.0
        pick = max(acc, key=lambda k: acc[k])
        acc[pick] -= 1.0
        out.append(pick)
    return out


_COLD_ENGINES = _cold_engines()


def _build_module():
    import concourse.tile as tile
    from concourse import bacc, mybir
    from contextlib import ExitStack

    f32 = mybir.dt.float32
    f16 = mybir.dt.float16
    f8 = mybir.dt.float8e4
    nc = bacc.Bacc("TRN2", target_bir_lowering=False, debug=False,
                   enable_asserts=True, num_devices=N_CORES)

    vsh_ap = nc.dram_tensor("vsh", (BLK, N), f16, kind="ExternalInput").ap()
    vsc_ap = nc.dram_tensor("vsc", (BLK, N), f16, kind="ExternalInput").ap()
    # per-quad bias columns, block 0 shipped first so early relu isn't gated
    ubh0_ap = nc.dram_tensor("ubh0", (BLK, 32), f32, kind="ExternalInput").ap()
    ubc0_ap = nc.dram_tensor("ubc0", (BLK, 32), f32, kind="ExternalInput").ap()
    ubh_ap = nc.dram_tensor("ubh", (BLK, N_QUADS - 32), f32, kind="ExternalInput").ap()
    ubc_ap = nc.dram_tensor("ubc", (BLK, N_QUADS - 32), f32, kind="ExternalInput").ap()
    # hot sign variants: 16 x [128, 64] f16; variant t has s_hot at rows
    # 32*d .. 32*d+31 of column 4t+d
    sgnh_ap = nc.dram_tensor("sgnh", (BLK, QUADS_PER_HALF * 64), f16,
                             kind="ExternalInput").ap()
    # DR sign variants: 8 x [128, 2, 64] f8
    sgnc_ap = nc.dram_tensor("sgnc", (BLK, OCTETS_PER_HALF * 128), f8,
                             kind="ExternalInput").ap()
    ident_ap = nc.dram_tensor("ident", (BLK, BLK), f16, kind="ExternalInput").ap()
    # adjwt / hpx host-permuted so each lands in one [128, *] SBUF tile:
    # adjwt[p, jb*512 + i] = w_j * adj[i, j],  j = jb*128 + p
    # hpx[p, jb*65 + n]    = [hp | 1][j, n],   j = jb*128 + p
    adjwt_ap = nc.dram_tensor("adjwt", (BLK, N_JB * ROWS_PER_CORE), f16,
                              kind="ExternalInput").ap()
    hpx_ap = nc.dram_tensor("hpx", (BLK, N_JB * (F + 1)), f16,
                            kind="ExternalInput").ap()
    out_ap = nc.dram_tensor("out", (ROWS_PER_CORE, F), f32, kind="ExternalOutput").ap()

    Relu = mybir.ActivationFunctionType.Relu
    Exp = mybir.ActivationFunctionType.Exp
    add = mybir.AluOpType.add
    amax = mybir.AluOpType.max
    amin = mybir.AluOpType.min
    mult = mybir.AluOpType.mult

    with tile.TileContext(nc) as tc, ExitStack() as ctx:
        consts = ctx.enter_context(tc.tile_pool(name="consts", bufs=1))
        hpool = ctx.enter_context(tc.tile_pool(name="hpool", bufs=6))
        cpool = ctx.enter_context(tc.tile_pool(name="cpool", bufs=4))
        epool = ctx.enter_context(tc.tile_pool(name="epool", bufs=2))
        apool = ctx.enter_context(tc.tile_pool(name="apool", bufs=3))
        spool = ctx.enter_context(tc.tile_pool(name="spool", bufs=4))
        ps_e = ctx.enter_context(tc.tile_pool(name="ps_e", bufs=2, space="PSUM"))
        ps_t = ctx.enter_context(tc.tile_pool(name="ps_t", bufs=3, space="PSUM"))
        ps_h = ctx.enter_context(tc.tile_pool(name="ps_h", bufs=1, space="PSUM"))

        # Spread the startup transfers over the SP/DVE/Pool DMA queues,
        # ordered by first need (vstacks -> first biases -> sign weights ->
        # late biases); ACT's queue stays free for its first productions.
        vsc = consts.tile([BLK, N], f16, tag="vsc")
        nc.sync.dma_start(vsc[:], vsc_ap[:])
        vsh = consts.tile([BLK, N], f16, tag="vsh")
        nc.scalar.dma_start(vsh[:], vsh_ap[:])
        ubc0 = consts.tile([BLK, 32], f32, tag="ubc0")
        nc.gpsimd.dma_start(ubc0[:], ubc0_ap[:])
        ubh0 = consts.tile([BLK, 32], f32, tag="ubh0")
        nc.gpsimd.dma_start(ubh0[:], ubh0_ap[:])
        sgnc = consts.tile([BLK, OCTETS_PER_HALF * 128], f8, tag="sgnc")
        nc.sync.dma_start(sgnc[:], sgnc_ap[:])
        sgnh = consts.tile([BLK, QUADS_PER_HALF * 64], f16, tag="sgnh")
        nc.scalar.dma_start(sgnh[:], sgnh_ap[:])
        ubh = consts.tile([BLK, N_QUADS - 32], f32, tag="ubh")
        nc.sync.dma_start(ubh[:], ubh_ap[:])
        ubc = consts.tile([BLK, N_QUADS - 32], f32, tag="ubc")
        nc.sync.dma_start(ubc[:], ubc_ap[:])
        adjwt = []
        hpx = []
        ident = []

        def load_aux():
            # issued after block 0's relu/matmul stream is underway so the
            # early compute waits don't entangle with these bulk transfers
            ident_t = consts.tile([BLK, BLK], f16, tag="ident")
            nc.gpsimd.dma_start(ident_t[:], ident_ap[:])
            ident.append(ident_t)
            adjwt_t = consts.tile([BLK, N_JB * ROWS_PER_CORE], f16, tag="adjwt")
            nc.gpsimd.dma_start(adjwt_t[:], adjwt_ap[:])
            hpx_t = consts.tile([BLK, N_JB * (F + 1)], f16, tag="hpx")
            nc.gpsimd.dma_start(hpx_t[:], hpx_ap[:])
            for jb in range(N_JB):
                adjwt.append(adjwt_t[:, jb * ROWS_PER_CORE:(jb + 1) * ROWS_PER_CORE])
                hpx.append(hpx_t[:, jb * (F + 1):(jb + 1) * (F + 1)])

        def hbias(q):
            return (ubh0[:, q:q + 1] if q < 32 else ubh[:, q - 32:q - 31])

        def cbias(q):
            return (ubc0[:, q:q + 1] if q < 32 else ubc[:, q - 32:q - 31])

        # DoubleRow matmuls must write PSUM at partition base 0 (walrus emits
        # full-array col_grp for them), so the kernel processes 64-dest
        # blocks: every matmul output (DR, hot, transpose, PV) is base-0.
        cold_idx = 0
        for blk in range(2 * N_BLOCKS):              # 8 blocks of 64 dests
            e_ps = ps_e.tile([64, N], f32, tag="e")
            for o in range(OCTETS_PER_HALF):
                qa = blk * 16 + 2 * o                # global quad ids
                qb = qa + 1
                cold8 = cpool.tile([BLK, 2 * N], f8, tag="cold")
                for half, q in ((0, qa), (1, qb)):
                    eng = getattr(nc, _COLD_ENGINES[cold_idx])
                    cold_idx += 1
                    dst = cold8[:, half * N:(half + 1) * N]
                    if eng is nc.scalar:
                        nc.scalar.activation(dst, vsc[:], Relu,
                                             bias=cbias(q), scale=1.0)
                    else:
                        eng.tensor_scalar(dst, vsc[:], cbias(q), 0.0,
                                          op0=add, op1=amax)
                hotA = hpool.tile([BLK, N], f16, tag="hot")
                nc.vector.tensor_scalar(hotA[:], vsh[:], hbias(qa), 0.0,
                                        op0=add, op1=amax)
                hotB = hpool.tile([BLK, N], f16, tag="hot")
                nc.vector.tensor_scalar(hotB[:], vsh[:], hbias(qb), 0.0,
                                        op0=add, op1=amax)
                rhs3 = cold8[:].rearrange("p (t n) -> p t n", t=2)
                lw_dr = (sgnc[:, o * 128:(o + 1) * 128]
                         .rearrange("p (t m) -> p t m", t=2))
                lw_a = sgnh[:, (2 * o) * 64:(2 * o) * 64 + 64]
                lw_b = sgnh[:, (2 * o + 1) * 64:(2 * o + 1) * 64 + 64]
                for jh in range(2):
                    sl = slice(jh * 512, jh * 512 + 512)
                    out_sl = e_ps[:, sl]
                    nc.tensor.matmul(
                        out_sl, lw_dr, rhs3[:, :, sl],
                        start=(o == 0), stop=False,
                        perf_mode=mybir.MatmulPerfMode.DoubleRow)
                    nc.tensor.matmul(out_sl, lw_a, hotA[:, sl],
                                     start=False, stop=False)
                    nc.tensor.matmul(out_sl, lw_b, hotB[:, sl],
                                     start=False,
                                     stop=(o == OCTETS_PER_HALF - 1))
            if blk == 0:
                load_aux()
            exp_sb = epool.tile([64, N], f16, tag="exp")
            nc.scalar.activation(exp_sb[:], e_ps[:], Exp, scale=(1.0 - ALPHA))
            hnum = ps_h.tile([64, F + 1], f32, tag="hnum")
            for jb in range(N_JB):
                tp = ps_t.tile([BLK, 64], f16, tag="tp")
                nc.tensor.transpose(tp[:], exp_sb[:, jb * BLK:(jb + 1) * BLK],
                                    ident[0][0:64, 0:64])
                attT = apool.tile([BLK, 64], f16, tag="attT")
                nc.vector.tensor_mul(
                    attT[:], tp[:], adjwt[jb][:, blk * 64:(blk + 1) * 64])
                nc.tensor.matmul(hnum[:], attT[:], hpx[jb],
                                 start=(jb == 0), stop=(jb == N_JB - 1))
            # epilogue: h = num/den, out = elu(h) = relu(h) + exp(min(h,0)) - 1
            rec = spool.tile([64, 1], f32, tag="rec")
            nc.vector.reciprocal(rec[:], hnum[:, F:F + 1])
            m_t = spool.tile([64, F], f32, tag="m_t")
            nc.vector.tensor_scalar(m_t[:], hnum[:, 0:F], rec[:, 0:1], 0.0,
                                    op0=mult, op1=amin)
            g_t = spool.tile([64, F], f32, tag="g_t")
            nc.scalar.activation(g_t[:], m_t[:], Exp)
            r_t = spool.tile([64, F], f32, tag="r_t")
            nc.vector.tensor_scalar(r_t[:], hnum[:, 0:F], rec[:, 0:1], 0.0,
                                    op0=mult, op1=amax)
            o2 = spool.tile([64, F], f32, tag="o2")
            nc.vector.scalar_tensor_tensor(
                o2[:], r_t[:], -1.0, g_t[:], op0=add, op1=add)
            nc.sync.dma_start(out_ap[blk * 64:(blk + 1) * 64, :], o2[:])

    nc.finalize()
    return nc


def _host_precompute(h, adj, lin_w, lin_b, W_w, a):
    """Build per-core device input dicts (all small math in float64)."""
    import ml_dtypes
    f8 = ml_dtypes.float8_e4m3

    h64 = h.astype(np.float64)
    lin_w64 = lin_w.astype(np.float64)
    lin_b64 = lin_b.astype(np.float64)
    W1 = W_w[:, :F].astype(np.float64)
    W2 = W_w[:, F:].astype(np.float64)
    a64 = a[:, 0].astype(np.float64)

    M1 = W1 @ lin_w64
    c1 = W1 @ lin_b64
    M2 = W2 @ lin_w64
    c2 = W2 @ lin_b64
    aab = np.abs(a64)
    sgn_vec = np.sign(a64)
    ident = np.eye(BLK, dtype=np.float16)

    in_maps = []
    for c in range(N_CORES):
        b = c // 2
        r0 = (c % 2) * ROWS_PER_CORE
        hb = h64[b]                                        # [N, F]
        u = (hb @ M1.T + c1) * aab                         # u'' [N, F]
        v = (hb @ M2.T + c2) * aab                         # v'' [N, F]
        sv = v @ sgn_vec                                   # [N]
        w = np.exp(ALPHA * sv)                             # [N]
        hp = hb @ lin_w64.T + lin_b64                      # [N, F]

        # feature split by second moment of u + v
        mom = u.var(0) + v.var(0) + (u.mean(0) + v.mean(0)) ** 2
        order = np.argsort(-mom)
        hot_f, cold_f = order[:HOT], order[HOT:]
        s_hot, s_cold = sgn_vec[hot_f], sgn_vec[cold_f]

        v16 = v.astype(np.float16)
        vsh = np.tile(v16[:, hot_f].T, (4, 1)).astype(np.float16)   # [128, N]
        vsc = np.tile(v16[:, cold_f].T, (4, 1)).astype(np.float16)  # [128, N]

        # per-quad bias columns: quad q covers dests r0 + 4q + d, d = row//32
        uc = u[r0:r0 + ROWS_PER_CORE]                      # [512, F]
        ubh = np.empty((BLK, N_QUADS), dtype=np.float32)
        ubc = np.empty((BLK, N_QUADS), dtype=np.float32)
        for d in range(4):
            ubh[d * 32:(d + 1) * 32, :] = uc[d::4, :][:, hot_f].T
            ubc[d * 32:(d + 1) * 32, :] = uc[d::4, :][:, cold_f].T

        # hot sign variants: 16 x [128, 64]
        sgnh = np.zeros((BLK, QUADS_PER_HALF, 64), dtype=np.float16)
        for t in range(QUADS_PER_HALF):
            for d in range(4):
                sgnh[d * 32:(d + 1) * 32, t, 4 * t + d] = s_hot
        sgnh = sgnh.reshape(BLK, QUADS_PER_HALF * 64)

        # DR sign variants: 8 x [128, 2, 64]
        sgnc = np.zeros((BLK, OCTETS_PER_HALF, 2, 64), dtype=f8)
        for o in range(OCTETS_PER_HALF):
            for k in range(2):
                for d in range(4):
                    sgnc[d * 32:(d + 1) * 32, o, k, 8 * o + 4 * k + d] = \
                        s_cold.astype(f8)
        sgnc = sgnc.reshape(BLK, OCTETS_PER_HALF * 128)

        adjwt = (adj[b, r0:r0 + ROWS_PER_CORE, :].T.astype(np.float64)
                 * w[:, None]).astype(np.float16)          # [N, 512]
        adjwt = adjwt.reshape(N_JB, BLK, ROWS_PER_CORE).transpose(1, 0, 2)
        adjwt = adjwt.reshape(BLK, N_JB * ROWS_PER_CORE)
        hpx = np.concatenate(
            [hp, np.ones((N, 1))], axis=1).astype(np.float16)  # [N, 65]
        hpx = hpx.reshape(N_JB, BLK, F + 1).transpose(1, 0, 2)
        hpx = hpx.reshape(BLK, N_JB * (F + 1))

        in_maps.append({
            "vsh": np.ascontiguousarray(vsh),
            "vsc": np.ascontiguousarray(vsc),
            "ubh0": np.ascontiguousarray(ubh[:, :32]),
            "ubc0": np.ascontiguousarray(ubc[:, :32]),
            "ubh": np.ascontiguousarray(ubh[:, 32:]),
            "ubc": np.ascontiguousarray(ubc[:, 32:]),
            "sgnh": sgnh,
            "sgnc": sgnc,
            "adjwt": np.ascontiguousarray(adjwt),
            "hpx": np.ascontiguousarray(hpx),
            "ident": ident,
        })
    return in_maps


def kernel(h, adj, lin_w, lin_b, W_w, a):
    from concourse.bass_utils import run_bass_kernel_spmd

    h, adj, lin_w, lin_b, W_w, a = (
        np.asarray(x) for x in (h, adj, lin_w, lin_b, W_w, a))

    if "nc" not in _COMPILED:
        _COMPILED["nc"] = _build_module()
    nc = _COMPILED["nc"]

    in_maps = _host_precompute(h, adj, lin_w, lin_b, W_w, a)
    res = run_bass_kernel_spmd(nc, in_maps, core_ids=list(range(N_CORES)))

    out = np.empty((B, N, F), dtype=np.float32)
    for c in range(N_CORES):
        b = c // 2
        r0 = (c % 2) * ROWS_PER_CORE
        out[b, r0:r0 + ROWS_PER_CORE, :] = res.results[c]["out"]
    return out


# revision 17
# speedup vs baseline: 1.4137x; 1.0263x over previous
"""GATv2 layer on 8 Trainium2 NeuronCores (Bass/Tile).

Reference math (per batch b):
    hp = h @ lin_w.T + lin_b
    u  = hp @ W1.T ; v = hp @ W2.T          (W1, W2 = halves of W_w)
    e[i,j]   = sum_f a_f * LeakyReLU(u[i,f] + v[j,f])
    att      = softmax_j(where(adj, e, -inf))
    out      = elu(att @ hp)

Kernel decomposition (same algebra as the fp16 predecessor):
  a_f*LReLU(s) = alpha*a_f*s + (1-alpha)*sign(a_f)*relu(|a_f|*s); with
  u'' = |a|*u, v'' = |a|*v the alpha*su_i row term cancels in softmax and
  exp(alpha*sv_j) folds into the adjacency mask host-side.  The remaining
  work per (i, j) is the 64-term signed-relu contraction
      c[i,j] = sum_f sign(a_f) * relu(u''[i,f] + v''[j,f]).

  Mixed-precision f-split: features are ranked host-side by the second
  moment of u''+v''; the top 32 ("hot") contract in fp16, the bottom 32
  ("cold") in fp8e4m3 via DoubleRow matmuls.  Per quad of destinations a
  [128, 1024] hot tile (4 dests x 32 f stacked on partitions) feeds one
  fp16 matmul per 512-wide j-half; per octet a [128, 2048] cold tile (two
  quad k-tiles) feeds one DoubleRow matmul ([128, 2, 512] moving operand,
  256-deep contraction).  Row-shifted +-sign weight variants place each
  group's rows inside the [64, 512] PSUM half (matmul output base
  partitions are restricted to {0, 64}), so 16 hot + 8 DR matmuls
  accumulate one e half.  Relative error vs the fp32 reference: ~4e-3
  (fp8 tail features), inside the 2e-2 gate.

  Tile production: relu(vstack + ubias_col) via tensor_scalar /
  activation, split across DVE (hot fp16, 4x mode), ACT and GPSIMD (cold
  fp8) so production overlaps the PE stream.  exp(0.8*e) via ACT, masked
  by w_j*adj^T during the PSUM->SBUF copy after a PE transpose, then the
  PV matmul (attT @ [hp, 1]) yields numerator and denominator in one
  pass; divide + ELU epilogue.

Sharding: core c owns batch c//2, destination rows (c%2)*512 ... +512.
"""

import sys

import numpy as np

if "/opt/trn_rl_repo" not in sys.path:
    sys.path.insert(0, "/opt/trn_rl_repo")

ALPHA = 0.2
B, N, F = 4, 1024, 64
N_CORES = 8
ROWS_PER_CORE = B * N // N_CORES          # 512
BLK = 128
N_BLOCKS = ROWS_PER_CORE // BLK           # 4
N_JB = N // BLK                           # 8
HOT = 32                                  # fp16 features
COLD = F - HOT                            # fp8 features
QUADS_PER_HALF = 16                       # 64 dests / 4
OCTETS_PER_HALF = 8
N_QUADS = ROWS_PER_CORE // 4              # 128

_COMPILED = {}


def _cold_engines():
    """Engines for the 128 cold production instrs: 60 ACT, 48 GP, 20 DVE,
    interleaved by largest remainder so each engine's share arrives evenly."""
    quotas = {"scalar": 58.0, "gpsimd": 46.0, "vector": 24.0}
    acc = dict.fromkeys(quotas, 0.0)
    out = []
    for _ in range(128):
        for k in quotas:
            acc[k] += quotas[k] / 128.0
        pick = max(acc, key=lambda k: acc[k])
        acc[pick] -= 1.0
        out.append(pick)
    # octet 0's pair must run in parallel on the two earliest-ready engines
    # (ACT + DVE); GPSIMD starts later
    i = out.index("vector")
    out[1], out[i] = out[i], out[1]
    return out


_COLD_ENGINES = _cold_engines()


def _build_module():
    import concourse.tile as tile
    from concourse import bacc, mybir
    from contextlib import ExitStack

    f32 = mybir.dt.float32
    f16 = mybir.dt.float16
    f8 = mybir.dt.float8e4
    nc = bacc.Bacc("TRN2", target_bir_lowering=False, debug=False,
                   enable_asserts=True, num_devices=N_CORES)

    vsh_ap = nc.dram_tensor("vsh", (BLK, N), f16, kind="ExternalInput").ap()
    vsc_ap = nc.dram_tensor("vsc", (BLK, N), f16, kind="ExternalInput").ap()
    # first 16 quads' bias columns (hot cols 0-15, cold 16-31) in one small
    # early transfer so the first productions aren't gated
    ub0_ap = nc.dram_tensor("ub0", (BLK, 32), f32, kind="ExternalInput").ap()
    ubh_ap = nc.dram_tensor("ubh", (BLK, N_QUADS - 16), f32, kind="ExternalInput").ap()
    ubc_ap = nc.dram_tensor("ubc", (BLK, N_QUADS - 16), f32, kind="ExternalInput").ap()
    # hot sign variants: 16 x [128, 64] f16; variant t has s_hot at rows
    # 32*d .. 32*d+31 of column 4t+d
    sgnh_ap = nc.dram_tensor("sgnh", (BLK, QUADS_PER_HALF * 64), f16,
                             kind="ExternalInput").ap()
    # DR sign variants: 8 x [128, 2, 64] f8
    sgnc_ap = nc.dram_tensor("sgnc", (BLK, OCTETS_PER_HALF * 128), f8,
                             kind="ExternalInput").ap()
    ident_ap = nc.dram_tensor("ident", (BLK, BLK), f16, kind="ExternalInput").ap()
    # adjwt / hpx host-permuted so each lands in one [128, *] SBUF tile:
    # adjwt[p, jb*512 + i] = w_j * adj[i, j],  j = jb*128 + p
    # hpx[p, jb*65 + n]    = [hp | 1][j, n],   j = jb*128 + p
    adjwt_ap = nc.dram_tensor("adjwt", (BLK, N_JB * ROWS_PER_CORE), f16,
                              kind="ExternalInput").ap()
    hpx_ap = nc.dram_tensor("hpx", (BLK, N_JB * (F + 1)), f16,
                            kind="ExternalInput").ap()
    out_ap = nc.dram_tensor("out", (ROWS_PER_CORE, F), f32, kind="ExternalOutput").ap()

    Relu = mybir.ActivationFunctionType.Relu
    Exp = mybir.ActivationFunctionType.Exp
    add = mybir.AluOpType.add
    amax = mybir.AluOpType.max
    amin = mybir.AluOpType.min
    mult = mybir.AluOpType.mult

    with tile.TileContext(nc) as tc, ExitStack() as ctx:
        consts = ctx.enter_context(tc.tile_pool(name="consts", bufs=1))
        hpool = ctx.enter_context(tc.tile_pool(name="hpool", bufs=6))
        cpool = ctx.enter_context(tc.tile_pool(name="cpool", bufs=4))
        epool = ctx.enter_context(tc.tile_pool(name="epool", bufs=2))
        apool = ctx.enter_context(tc.tile_pool(name="apool", bufs=3))
        spool = ctx.enter_context(tc.tile_pool(name="spool", bufs=4))
        ps_e = ctx.enter_context(tc.tile_pool(name="ps_e", bufs=2, space="PSUM"))
        ps_t = ctx.enter_context(tc.tile_pool(name="ps_t", bufs=3, space="PSUM"))
        ps_h = ctx.enter_context(tc.tile_pool(name="ps_h", bufs=1, space="PSUM"))

        # Startup transfers: the tiny first-bias block rides the SWDGE
        # (gpsimd) path, bypassing the shared HWDGE generator; the rest
        # alternate between the SP and ACT HWDGE queues ordered by first
        # need (vstacks -> sign weights -> late biases).
        ub0 = consts.tile([BLK, 32], f32, tag="ub0")
        nc.gpsimd.dma_start(ub0[:], ub0_ap[:])
        vsc = consts.tile([BLK, N], f16, tag="vsc")
        nc.sync.dma_start(vsc[:], vsc_ap[:])
        vsh = consts.tile([BLK, N], f16, tag="vsh")
        nc.scalar.dma_start(vsh[:], vsh_ap[:])
        sgnc = consts.tile([BLK, OCTETS_PER_HALF * 128], f8, tag="sgnc")
        nc.sync.dma_start(sgnc[:], sgnc_ap[:])
        sgnh = consts.tile([BLK, QUADS_PER_HALF * 64], f16, tag="sgnh")
        nc.scalar.dma_start(sgnh[:], sgnh_ap[:])
        ubh = consts.tile([BLK, N_QUADS - 16], f32, tag="ubh")
        nc.sync.dma_start(ubh[:], ubh_ap[:])
        ubc = consts.tile([BLK, N_QUADS - 16], f32, tag="ubc")
        nc.scalar.dma_start(ubc[:], ubc_ap[:])
        adjwt = []
        hpx = []
        ident = []

        def load_aux():
            # issued after block 0's relu/matmul stream is underway so the
            # early compute waits don't entangle with these bulk transfers
            # (HWDGE queues, so the Pool compute engine stays free)
            ident_t = consts.tile([BLK, BLK], f16, tag="ident")
            nc.sync.dma_start(ident_t[:], ident_ap[:])
            ident.append(ident_t)
            adjwt_t = consts.tile([BLK, N_JB * ROWS_PER_CORE], f16, tag="adjwt")
            nc.sync.dma_start(adjwt_t[:], adjwt_ap[:])
            hpx_t = consts.tile([BLK, N_JB * (F + 1)], f16, tag="hpx")
            nc.scalar.dma_start(hpx_t[:], hpx_ap[:])
            for jb in range(N_JB):
                adjwt.append(adjwt_t[:, jb * ROWS_PER_CORE:(jb + 1) * ROWS_PER_CORE])
                hpx.append(hpx_t[:, jb * (F + 1):(jb + 1) * (F + 1)])

        def hbias(q):
            return (ub0[:, q:q + 1] if q < 16 else ubh[:, q - 16:q - 15])

        def cbias(q):
            return (ub0[:, 16 + q:17 + q] if q < 16 else ubc[:, q - 16:q - 15])

        # DoubleRow matmuls must write PSUM at partition base 0 (walrus emits
        # full-array col_grp for them), so the kernel processes 64-dest
        # blocks: every matmul output (DR, hot, transpose, PV) is base-0.
        cold_idx = 0
        for blk in range(2 * N_BLOCKS):              # 8 blocks of 64 dests
            e_ps = ps_e.tile([64, N], f32, tag="e")
            for o in range(OCTETS_PER_HALF):
                qa = blk * 16 + 2 * o                # global quad ids
                qb = qa + 1
                cold8 = cpool.tile([BLK, 2 * N], f8, tag="cold")
                for half, q in ((0, qa), (1, qb)):
                    eng = getattr(nc, _COLD_ENGINES[cold_idx])
                    cold_idx += 1
                    dst = cold8[:, half * N:(half + 1) * N]
                    if eng is nc.scalar:
                        nc.scalar.activation(dst, vsc[:], Relu,
                                             bias=cbias(q), scale=1.0)
                    else:
                        eng.tensor_scalar(dst, vsc[:], cbias(q), 0.0,
                                          op0=add, op1=amax)
                hotA = hpool.tile([BLK, N], f16, tag="hot")
                nc.vector.tensor_scalar(hotA[:], vsh[:], hbias(qa), 0.0,
                                        op0=add, op1=amax)
                hotB = hpool.tile([BLK, N], f16, tag="hot")
                nc.vector.tensor_scalar(hotB[:], vsh[:], hbias(qb), 0.0,
                                        op0=add, op1=amax)
                rhs3 = cold8[:].rearrange("p (t n) -> p t n", t=2)
                lw_dr = (sgnc[:, o * 128:(o + 1) * 128]
                         .rearrange("p (t m) -> p t m", t=2))
                lw_a = sgnh[:, (2 * o) * 64:(2 * o) * 64 + 64]
                lw_b = sgnh[:, (2 * o + 1) * 64:(2 * o + 1) * 64 + 64]
                for jh in range(2):
                    sl = slice(jh * 512, jh * 512 + 512)
                    out_sl = e_ps[:, sl]
                    nc.tensor.matmul(
                        out_sl, lw_dr, rhs3[:, :, sl],
                        start=(o == 0), stop=False,
                        perf_mode=mybir.MatmulPerfMode.DoubleRow)
                    nc.tensor.matmul(out_sl, lw_a, hotA[:, sl],
                                     start=False, stop=False)
                    nc.tensor.matmul(out_sl, lw_b, hotB[:, sl],
                                     start=False,
                                     stop=(o == OCTETS_PER_HALF - 1))
            if blk == 0:
                load_aux()
            # exp((1-alpha) * e), split in column halves so the first
            # transposes are not gated on the full pass
            exp_sb = epool.tile([64, N], f16, tag="exp")
            nc.scalar.activation(exp_sb[:, 0:512], e_ps[:, 0:512], Exp,
                                 scale=(1.0 - ALPHA))
            nc.scalar.activation(exp_sb[:, 512:1024], e_ps[:, 512:1024], Exp,
                                 scale=(1.0 - ALPHA))
            hnum = ps_h.tile([64, F + 1], f32, tag="hnum")
            for jb in range(N_JB):
                tp = ps_t.tile([BLK, 64], f16, tag="tp")
                nc.tensor.transpose(tp[:], exp_sb[:, jb * BLK:(jb + 1) * BLK],
                                    ident[0][0:64, 0:64])
                attT = apool.tile([BLK, 64], f16, tag="attT")
                nc.vector.tensor_mul(
                    attT[:], tp[:], adjwt[jb][:, blk * 64:(blk + 1) * 64])
                nc.tensor.matmul(hnum[:], attT[:], hpx[jb],
                                 start=(jb == 0), stop=(jb == N_JB - 1))
            # epilogue: h = num/den, out = elu(h) = relu(h) + exp(min(h,0)) - 1
            rec = spool.tile([64, 1], f32, tag="rec")
            nc.vector.reciprocal(rec[:], hnum[:, F:F + 1])
            m_t = spool.tile([64, F], f32, tag="m_t")
            nc.vector.tensor_scalar(m_t[:], hnum[:, 0:F], rec[:, 0:1], 0.0,
                                    op0=mult, op1=amin)
            g_t = spool.tile([64, F], f32, tag="g_t")
            nc.scalar.activation(g_t[:], m_t[:], Exp)
            r_t = spool.tile([64, F], f32, tag="r_t")
            nc.vector.tensor_scalar(r_t[:], hnum[:, 0:F], rec[:, 0:1], 0.0,
                                    op0=mult, op1=amax)
            o2 = spool.tile([64, F], f32, tag="o2")
            nc.vector.scalar_tensor_tensor(
                o2[:], r_t[:], -1.0, g_t[:], op0=add, op1=add)
            nc.sync.dma_start(out_ap[blk * 64:(blk + 1) * 64, :], o2[:])

    nc.finalize()
    return nc


def _host_precompute(h, adj, lin_w, lin_b, W_w, a):
    """Build per-core device input dicts (all small math in float64)."""
    import ml_dtypes
    f8 = ml_dtypes.float8_e4m3

    h64 = h.astype(np.float64)
    lin_w64 = lin_w.astype(np.float64)
    lin_b64 = lin_b.astype(np.float64)
    W1 = W_w[:, :F].astype(np.float64)
    W2 = W_w[:, F:].astype(np.float64)
    a64 = a[:, 0].astype(np.float64)

    M1 = W1 @ lin_w64
    c1 = W1 @ lin_b64
    M2 = W2 @ lin_w64
    c2 = W2 @ lin_b64
    aab = np.abs(a64)
    sgn_vec = np.sign(a64)
    ident = np.eye(BLK, dtype=np.float16)

    in_maps = []
    for c in range(N_CORES):
        b = c // 2
        r0 = (c % 2) * ROWS_PER_CORE
        hb = h64[b]                                        # [N, F]
        u = (hb @ M1.T + c1) * aab                         # u'' [N, F]
        v = (hb @ M2.T + c2) * aab                         # v'' [N, F]
        sv = v @ sgn_vec                                   # [N]
        w = np.exp(ALPHA * sv)                             # [N]
        hp = hb @ lin_w64.T + lin_b64                      # [N, F]

        # feature split by second moment of u + v
        mom = u.var(0) + v.var(0) + (u.mean(0) + v.mean(0)) ** 2
        order = np.argsort(-mom)
        hot_f, cold_f = order[:HOT], order[HOT:]
        s_hot, s_cold = sgn_vec[hot_f], sgn_vec[cold_f]

        v16 = v.astype(np.float16)
        vsh = np.tile(v16[:, hot_f].T, (4, 1)).astype(np.float16)   # [128, N]
        vsc = np.tile(v16[:, cold_f].T, (4, 1)).astype(np.float16)  # [128, N]

        # per-quad bias columns: quad q covers dests r0 + 4q + d, d = row//32
        uc = u[r0:r0 + ROWS_PER_CORE]                      # [512, F]
        ubh = np.empty((BLK, N_QUADS), dtype=np.float32)
        ubc = np.empty((BLK, N_QUADS), dtype=np.float32)
        for d in range(4):
            ubh[d * 32:(d + 1) * 32, :] = uc[d::4, :][:, hot_f].T
            ubc[d * 32:(d + 1) * 32, :] = uc[d::4, :][:, cold_f].T

        # hot sign variants: 16 x [128, 64]
        sgnh = np.zeros((BLK, QUADS_PER_HALF, 64), dtype=np.float16)
        for t in range(QUADS_PER_HALF):
            for d in range(4):
                sgnh[d * 32:(d + 1) * 32, t, 4 * t + d] = s_hot
        sgnh = sgnh.reshape(BLK, QUADS_PER_HALF * 64)

        # DR sign variants: 8 x [128, 2, 64]
        sgnc = np.zeros((BLK, OCTETS_PER_HALF, 2, 64), dtype=f8)
        for o in range(OCTETS_PER_HALF):
            for k in range(2):
                for d in range(4):
                    sgnc[d * 32:(d + 1) * 32, o, k, 8 * o + 4 * k + d] = \
                        s_cold.astype(f8)
        sgnc = sgnc.reshape(BLK, OCTETS_PER_HALF * 128)

        adjwt = (adj[b, r0:r0 + ROWS_PER_CORE, :].T.astype(np.float64)
                 * w[:, None]).astype(np.float16)          # [N, 512]
        adjwt = adjwt.reshape(N_JB, BLK, ROWS_PER_CORE).transpose(1, 0, 2)
        adjwt = adjwt.reshape(BLK, N_JB * ROWS_PER_CORE)
        hpx = np.concatenate(
            [hp, np.ones((N, 1))], axis=1).astype(np.float16)  # [N, 65]
        hpx = hpx.reshape(N_JB, BLK, F + 1).transpose(1, 0, 2)
        hpx = hpx.reshape(BLK, N_JB * (F + 1))

        in_maps.append({
            "vsh": np.ascontiguousarray(vsh),
            "vsc": np.ascontiguousarray(vsc),
            "ub0": np.ascontiguousarray(
                np.concatenate([ubh[:, :16], ubc[:, :16]], axis=1)),
            "ubh": np.ascontiguousarray(ubh[:, 16:]),
            "ubc": np.ascontiguousarray(ubc[:, 16:]),
            "sgnh": sgnh,
            "sgnc": sgnc,
            "adjwt": np.ascontiguousarray(adjwt),
            "hpx": np.ascontiguousarray(hpx),
            "ident": ident,
        })
    return in_maps


def kernel(h, adj, lin_w, lin_b, W_w, a):
    from concourse.bass_utils import run_bass_kernel_spmd

    h, adj, lin_w, lin_b, W_w, a = (
        np.asarray(x) for x in (h, adj, lin_w, lin_b, W_w, a))

    if "nc" not in _COMPILED:
        _COMPILED["nc"] = _build_module()
    nc = _COMPILED["nc"]

    in_maps = _host_precompute(h, adj, lin_w, lin_b, W_w, a)
    res = run_bass_kernel_spmd(nc, in_maps, core_ids=list(range(N_CORES)))

    out = np.empty((B, N, F), dtype=np.float32)
    for c in range(N_CORES):
        b = c // 2
        r0 = (c % 2) * ROWS_PER_CORE
        out[b, r0:r0 + ROWS_PER_CORE, :] = res.results[c]["out"]
    return out


# revision 23
# speedup vs baseline: 1.4158x; 1.0015x over previous
"""GATv2 layer on 8 Trainium2 NeuronCores (Bass/Tile).

Reference math (per batch b):
    hp = h @ lin_w.T + lin_b
    u  = hp @ W1.T ; v = hp @ W2.T          (W1, W2 = halves of W_w)
    e[i,j]   = sum_f a_f * LeakyReLU(u[i,f] + v[j,f])
    att      = softmax_j(where(adj, e, -inf))
    out      = elu(att @ hp)

Kernel decomposition (same algebra as the fp16 predecessor):
  a_f*LReLU(s) = alpha*a_f*s + (1-alpha)*sign(a_f)*relu(|a_f|*s); with
  u'' = |a|*u, v'' = |a|*v the alpha*su_i row term cancels in softmax and
  exp(alpha*sv_j) folds into the adjacency mask host-side.  The remaining
  work per (i, j) is the 64-term signed-relu contraction
      c[i,j] = sum_f sign(a_f) * relu(u''[i,f] + v''[j,f]).

  Mixed-precision f-split: features are ranked host-side by the second
  moment of u''+v''; the top 32 ("hot") contract in fp16, the bottom 32
  ("cold") in fp8e4m3 via DoubleRow matmuls.  Per quad of destinations a
  [128, 1024] hot tile (4 dests x 32 f stacked on partitions) feeds one
  fp16 matmul per 512-wide j-half; per octet a [128, 2048] cold tile (two
  quad k-tiles) feeds one DoubleRow matmul ([128, 2, 512] moving operand,
  256-deep contraction).  Row-shifted +-sign weight variants place each
  group's rows inside the [64, 512] PSUM half (matmul output base
  partitions are restricted to {0, 64}), so 16 hot + 8 DR matmuls
  accumulate one e half.  Relative error vs the fp32 reference: ~4e-3
  (fp8 tail features), inside the 2e-2 gate.

  Tile production: relu(vstack + ubias_col) via tensor_scalar /
  activation, split across DVE (hot fp16, 4x mode), ACT and GPSIMD (cold
  fp8) so production overlaps the PE stream.  exp(0.8*e) via ACT, masked
  by w_j*adj^T during the PSUM->SBUF copy after a PE transpose, then the
  PV matmul (attT @ [hp, 1]) yields numerator and denominator in one
  pass; divide + ELU epilogue.

Sharding: core c owns batch c//2, destination rows (c%2)*512 ... +512.
"""

import sys

import numpy as np

if "/opt/trn_rl_repo" not in sys.path:
    sys.path.insert(0, "/opt/trn_rl_repo")

ALPHA = 0.2
B, N, F = 4, 1024, 64
N_CORES = 8
ROWS_PER_CORE = B * N // N_CORES          # 512
BLK = 128
N_BLOCKS = ROWS_PER_CORE // BLK           # 4
N_JB = N // BLK                           # 8
HOT = 32                                  # fp16 features
COLD = F - HOT                            # fp8 features
QUADS_PER_HALF = 16                       # 64 dests / 4
OCTETS_PER_HALF = 8
N_QUADS = ROWS_PER_CORE // 4              # 128

_COMPILED = {}


def _cold_engines():
    """Engines for the 128 cold production instrs: 60 ACT, 48 GP, 20 DVE,
    interleaved by largest remainder so each engine's share arrives evenly."""
    quotas = {"scalar": 58.0, "gpsimd": 46.0, "vector": 24.0}
    acc = dict.fromkeys(quotas, 0.0)
    out = []
    for _ in range(128):
        for k in quotas:
            acc[k] += quotas[k] / 128.0
        pick = max(acc, key=lambda k: acc[k])
        acc[pick] -= 1.0
        out.append(pick)
    # octet 0's pair must run in parallel on the two earliest-ready engines
    # (ACT + DVE); GPSIMD starts later
    i = out.index("vector")
    out[1], out[i] = out[i], out[1]
    return out


_COLD_ENGINES = _cold_engines()


def _build_module():
    import concourse.tile as tile
    from concourse import bacc, mybir
    from contextlib import ExitStack

    f32 = mybir.dt.float32
    f16 = mybir.dt.float16
    f8 = mybir.dt.float8e4
    nc = bacc.Bacc("TRN2", target_bir_lowering=False, debug=False,
                   enable_asserts=True, num_devices=N_CORES)

    # merged vstacks (hot cols 0:1024, cold 1024:2048) — one HWDGE slot
    vs_ap = nc.dram_tensor("vs", (BLK, 2 * N), f16, kind="ExternalInput").ap()
    # first 16 quads' bias columns (hot cols 0-15, cold 16-31) in one small
    # early transfer so the first productions aren't gated
    ub0_ap = nc.dram_tensor("ub0", (BLK, 32), f32, kind="ExternalInput").ap()
    # merged late biases: hot quads 16-127 at cols 0:112, cold at 112:224
    ubhc_ap = nc.dram_tensor("ubhc", (BLK, 2 * (N_QUADS - 16)), f32,
                             kind="ExternalInput").ap()
    # hot sign variants: 16 x [128, 64] f16; variant t has s_hot at rows
    # 32*d .. 32*d+31 of column 4t+d
    sgnh_ap = nc.dram_tensor("sgnh", (BLK, QUADS_PER_HALF * 64), f16,
                             kind="ExternalInput").ap()
    # DR sign variants: 8 x [128, 2, 64] f8
    sgnc_ap = nc.dram_tensor("sgnc", (BLK, OCTETS_PER_HALF * 128), f8,
                             kind="ExternalInput").ap()
    ident_ap = nc.dram_tensor("ident", (BLK, BLK), f16, kind="ExternalInput").ap()
    # adjwt / hpx host-permuted so each lands in one [128, *] SBUF tile:
    # adjwt[p, jb*512 + i] = w_j * adj[i, j],  j = jb*128 + p
    # hpx[p, jb*65 + n]    = [hp | 1][j, n],   j = jb*128 + p
    adjwt_ap = nc.dram_tensor("adjwt", (BLK, N_JB * ROWS_PER_CORE), f16,
                              kind="ExternalInput").ap()
    hpx_ap = nc.dram_tensor("hpx", (BLK, N_JB * (F + 1)), f16,
                            kind="ExternalInput").ap()
    out_ap = nc.dram_tensor("out", (ROWS_PER_CORE, F), f32, kind="ExternalOutput").ap()

    Relu = mybir.ActivationFunctionType.Relu
    Exp = mybir.ActivationFunctionType.Exp
    add = mybir.AluOpType.add
    amax = mybir.AluOpType.max
    amin = mybir.AluOpType.min
    mult = mybir.AluOpType.mult

    with tile.TileContext(nc) as tc, ExitStack() as ctx:
        consts = ctx.enter_context(tc.tile_pool(name="consts", bufs=1))
        hpool = ctx.enter_context(tc.tile_pool(name="hpool", bufs=6))
        cpool = ctx.enter_context(tc.tile_pool(name="cpool", bufs=4))
        epool = ctx.enter_context(tc.tile_pool(name="epool", bufs=2))
        apool = ctx.enter_context(tc.tile_pool(name="apool", bufs=3))
        spool = ctx.enter_context(tc.tile_pool(name="spool", bufs=4))
        ps_e = ctx.enter_context(tc.tile_pool(name="ps_e", bufs=2, space="PSUM"))
        ps_t = ctx.enter_context(tc.tile_pool(name="ps_t", bufs=3, space="PSUM"))
        ps_h = ctx.enter_context(tc.tile_pool(name="ps_h", bufs=1, space="PSUM"))

        # Startup transfers. A DMA instruction holds its queue's sequencer
        # until the transfer's semaphore fires (~2.3us each), so each queue
        # carries exactly one early transfer: merged vstacks on SP, hot sign
        # weights on ACT, and the small/late ones on the SWDGE (gpsimd) path
        # which only costs Pool-engine descriptor generation.
        ub0 = consts.tile([BLK, 32], f32, tag="ub0")
        nc.gpsimd.dma_start(ub0[:], ub0_ap[:])
        vs = consts.tile([BLK, 2 * N], f16, tag="vs")
        nc.sync.dma_start(vs[:], vs_ap[:])
        vsh = vs[:, 0:N]
        vsc = vs[:, N:2 * N]
        sgnh = consts.tile([BLK, QUADS_PER_HALF * 64], f16, tag="sgnh")
        nc.scalar.dma_start(sgnh[:], sgnh_ap[:])
        ident_t = consts.tile([BLK, BLK], f16, tag="ident")
        nc.gpsimd.dma_start(ident_t[:], ident_ap[:])
        sgnc = consts.tile([BLK, OCTETS_PER_HALF * 128], f8, tag="sgnc")
        nc.gpsimd.dma_start(sgnc[:], sgnc_ap[:])
        ubhc = consts.tile([BLK, 2 * (N_QUADS - 16)], f32, tag="ubhc")
        nc.sync.dma_start(ubhc[:], ubhc_ap[:])
        adjwt = []
        hpx = []
        ident = []

        def load_aux():
            # issued after block 0's relu/matmul stream is underway so the
            # early compute waits don't entangle with these bulk transfers
            # (SP + SWDGE; the ACT queue stays free for productions)
            adjwt_t = consts.tile([BLK, N_JB * ROWS_PER_CORE], f16, tag="adjwt")
            nc.sync.dma_start(adjwt_t[:], adjwt_ap[:])
            hpx_t = consts.tile([BLK, N_JB * (F + 1)], f16, tag="hpx")
            nc.gpsimd.dma_start(hpx_t[:], hpx_ap[:])
            for jb in range(N_JB):
                adjwt.append(adjwt_t[:, jb * ROWS_PER_CORE:(jb + 1) * ROWS_PER_CORE])
                hpx.append(hpx_t[:, jb * (F + 1):(jb + 1) * (F + 1)])

        ident.append(ident_t)
        NQ16 = N_QUADS - 16

        def hbias(q):
            return (ub0[:, q:q + 1] if q < 16 else ubhc[:, q - 16:q - 15])

        def cbias(q):
            return (ub0[:, 16 + q:17 + q] if q < 16
                    else ubhc[:, NQ16 + q - 16:NQ16 + q - 15])

        # DoubleRow matmuls must write PSUM at partition base 0 (walrus emits
        # full-array col_grp for them), so the kernel processes 64-dest
        # blocks: every matmul output (DR, hot, transpose, PV) is base-0.
        cold_idx = 0
        for blk in range(2 * N_BLOCKS):              # 8 blocks of 64 dests
            e_ps = ps_e.tile([64, N], f32, tag="e")
            for o in range(OCTETS_PER_HALF):
                qa = blk * 16 + 2 * o                # global quad ids
                qb = qa + 1
                cold8 = cpool.tile([BLK, 2 * N], f8, tag="cold")
                for half, q in ((0, qa), (1, qb)):
                    eng = getattr(nc, _COLD_ENGINES[cold_idx])
                    cold_idx += 1
                    dst = cold8[:, half * N:(half + 1) * N]
                    if eng is nc.scalar:
                        nc.scalar.activation(dst, vsc[:], Relu,
                                             bias=cbias(q), scale=1.0)
                    else:
                        eng.tensor_scalar(dst, vsc[:], cbias(q), 0.0,
                                          op0=add, op1=amax)
                hotA = hpool.tile([BLK, N], f16, tag="hot")
                nc.vector.tensor_scalar(hotA[:], vsh[:], hbias(qa), 0.0,
                                        op0=add, op1=amax)
                hotB = hpool.tile([BLK, N], f16, tag="hot")
                nc.vector.tensor_scalar(hotB[:], vsh[:], hbias(qb), 0.0,
                                        op0=add, op1=amax)
                rhs3 = cold8[:].rearrange("p (t n) -> p t n", t=2)
                lw_dr = (sgnc[:, o * 128:(o + 1) * 128]
                         .rearrange("p (t m) -> p t m", t=2))
                lw_a = sgnh[:, (2 * o) * 64:(2 * o) * 64 + 64]
                lw_b = sgnh[:, (2 * o + 1) * 64:(2 * o + 1) * 64 + 64]
                for jh in range(2):
                    sl = slice(jh * 512, jh * 512 + 512)
                    out_sl = e_ps[:, sl]
                    nc.tensor.matmul(out_sl, lw_a, hotA[:, sl],
                                     start=(o == 0), stop=False)
                    nc.tensor.matmul(out_sl, lw_b, hotB[:, sl],
                                     start=False, stop=False)
                    nc.tensor.matmul(
                        out_sl, lw_dr, rhs3[:, :, sl],
                        start=False, stop=(o == OCTETS_PER_HALF - 1),
                        perf_mode=mybir.MatmulPerfMode.DoubleRow)
            if blk == 0:
                load_aux()
            # exp((1-alpha) * e), split in column halves so the first
            # transposes are not gated on the full pass
            exp_sb = epool.tile([64, N], f16, tag="exp")
            nc.scalar.activation(exp_sb[:, 0:512], e_ps[:, 0:512], Exp,
                                 scale=(1.0 - ALPHA))
            nc.scalar.activation(exp_sb[:, 512:1024], e_ps[:, 512:1024], Exp,
                                 scale=(1.0 - ALPHA))
            hnum = ps_h.tile([64, F + 1], f32, tag="hnum")
            for jb in range(N_JB):
                tp = ps_t.tile([BLK, 64], f16, tag="tp")
                nc.tensor.transpose(tp[:], exp_sb[:, jb * BLK:(jb + 1) * BLK],
                                    ident[0][0:64, 0:64])
                attT = apool.tile([BLK, 64], f16, tag="attT")
                nc.vector.tensor_mul(
                    attT[:], tp[:], adjwt[jb][:, blk * 64:(blk + 1) * 64])
                nc.tensor.matmul(hnum[:], attT[:], hpx[jb],
                                 start=(jb == 0), stop=(jb == N_JB - 1))
            # epilogue: h = num/den, out = elu(h) = relu(h) + exp(min(h,0)) - 1
            rec = spool.tile([64, 1], f32, tag="rec")
            nc.vector.reciprocal(rec[:], hnum[:, F:F + 1])
            m_t = spool.tile([64, F], f32, tag="m_t")
            nc.vector.tensor_scalar(m_t[:], hnum[:, 0:F], rec[:, 0:1], 0.0,
                                    op0=mult, op1=amin)
            g_t = spool.tile([64, F], f32, tag="g_t")
            nc.scalar.activation(g_t[:], m_t[:], Exp)
            r_t = spool.tile([64, F], f32, tag="r_t")
            nc.vector.tensor_scalar(r_t[:], hnum[:, 0:F], rec[:, 0:1], 0.0,
                                    op0=mult, op1=amax)
            o2 = spool.tile([64, F], f32, tag="o2")
            nc.vector.scalar_tensor_tensor(
                o2[:], r_t[:], -1.0, g_t[:], op0=add, op1=add)
            nc.sync.dma_start(out_ap[blk * 64:(blk + 1) * 64, :], o2[:])

    nc.finalize()
    return nc


def _host_precompute(h, adj, lin_w, lin_b, W_w, a):
    """Build per-core device input dicts (all small math in float64)."""
    import ml_dtypes
    f8 = ml_dtypes.float8_e4m3

    h64 = h.astype(np.float64)
    lin_w64 = lin_w.astype(np.float64)
    lin_b64 = lin_b.astype(np.float64)
    W1 = W_w[:, :F].astype(np.float64)
    W2 = W_w[:, F:].astype(np.float64)
    a64 = a[:, 0].astype(np.float64)

    M1 = W1 @ lin_w64
    c1 = W1 @ lin_b64
    M2 = W2 @ lin_w64
    c2 = W2 @ lin_b64
    aab = np.abs(a64)
    sgn_vec = np.sign(a64)
    ident = np.eye(BLK, dtype=np.float16)

    in_maps = []
    for c in range(N_CORES):
        b = c // 2
        r0 = (c % 2) * ROWS_PER_CORE
        hb = h64[b]                                        # [N, F]
        u = (hb @ M1.T + c1) * aab                         # u'' [N, F]
        v = (hb @ M2.T + c2) * aab                         # v'' [N, F]
        sv = v @ sgn_vec                                   # [N]
        w = np.exp(ALPHA * sv)                             # [N]
        hp = hb @ lin_w64.T + lin_b64                      # [N, F]

        # feature split by second moment of u + v
        mom = u.var(0) + v.var(0) + (u.mean(0) + v.mean(0)) ** 2
        order = np.argsort(-mom)
        hot_f, cold_f = order[:HOT], order[HOT:]
        s_hot, s_cold = sgn_vec[hot_f], sgn_vec[cold_f]

        v16 = v.astype(np.float16)
        vsh = np.tile(v16[:, hot_f].T, (4, 1)).astype(np.float16)   # [128, N]
        vsc = np.tile(v16[:, cold_f].T, (4, 1)).astype(np.float16)  # [128, N]

        # per-quad bias columns: quad q covers dests r0 + 4q + d, d = row//32
        uc = u[r0:r0 + ROWS_PER_CORE]                      # [512, F]
        ubh = np.empty((BLK, N_QUADS), dtype=np.float32)
        ubc = np.empty((BLK, N_QUADS), dtype=np.float32)
        for d in range(4):
            ubh[d * 32:(d + 1) * 32, :] = uc[d::4, :][:, hot_f].T
            ubc[d * 32:(d + 1) * 32, :] = uc[d::4, :][:, cold_f].T

        # hot sign variants: 16 x [128, 64]
        sgnh = np.zeros((BLK, QUADS_PER_HALF, 64), dtype=np.float16)
        for t in range(QUADS_PER_HALF):
            for d in range(4):
                sgnh[d * 32:(d + 1) * 32, t, 4 * t + d] = s_hot
        sgnh = sgnh.reshape(BLK, QUADS_PER_HALF * 64)

        # DR sign variants: 8 x [128, 2, 64]
        sgnc = np.zeros((BLK, OCTETS_PER_HALF, 2, 64), dtype=f8)
        for o in range(OCTETS_PER_HALF):
            for k in range(2):
                for d in range(4):
                    sgnc[d * 32:(d + 1) * 32, o, k, 8 * o + 4 * k + d] = \
                        s_cold.astype(f8)
        sgnc = sgnc.reshape(BLK, OCTETS_PER_HALF * 128)

        adjwt = (adj[b, r0:r0 + ROWS_PER_CORE, :].T.astype(np.float64)
                 * w[:, None]).astype(np.float16)          # [N, 512]
        adjwt = adjwt.reshape(N_JB, BLK, ROWS_PER_CORE).transpose(1, 0, 2)
        adjwt = adjwt.reshape(BLK, N_JB * ROWS_PER_CORE)
        hpx = np.concatenate(
            [hp, np.ones((N, 1))], axis=1).astype(np.float16)  # [N, 65]
        hpx = hpx.reshape(N_JB, BLK, F + 1).transpose(1, 0, 2)
        hpx = hpx.reshape(BLK, N_JB * (F + 1))

        in_maps.append({
            "vs": np.ascontiguousarray(
                np.concatenate([vsh, vsc], axis=1)),
            "ub0": np.ascontiguousarray(
                np.concatenate([ubh[:, :16], ubc[:, :16]], axis=1)),
            "ubhc": np.ascontiguousarray(
                np.concatenate([ubh[:, 16:], ubc[:, 16:]], axis=1)),
            "sgnh": sgnh,
            "sgnc": sgnc,
            "adjwt": np.ascontiguousarray(adjwt),
            "hpx": np.ascontiguousarray(hpx),
            "ident": ident,
        })
    return in_maps


def kernel(h, adj, lin_w, lin_b, W_w, a):
    from concourse.bass_utils import run_bass_kernel_spmd

    h, adj, lin_w, lin_b, W_w, a = (
        np.asarray(x) for x in (h, adj, lin_w, lin_b, W_w, a))

    if "nc" not in _COMPILED:
        _COMPILED["nc"] = _build_module()
    nc = _COMPILED["nc"]

    in_maps = _host_precompute(h, adj, lin_w, lin_b, W_w, a)
    res = run_bass_kernel_spmd(nc, in_maps, core_ids=list(range(N_CORES)))

    out = np.empty((B, N, F), dtype=np.float32)
    for c in range(N_CORES):
        b = c // 2
        r0 = (c % 2) * ROWS_PER_CORE
        out[b, r0:r0 + ROWS_PER_CORE, :] = res.results[c]["out"]
    return out
